# revision 51
# baseline (speedup 1.0000x reference)
"""Trainium2 Bass kernel for nn_AGCnet — 8-core batch-parallel.

Reference structure (B=16, C=64, H=W=256):
  x0  = AdaptiveAvgPool2d((2,2))(x)                      [B,C,2,2]
  x0  = conv3x3(x0, w1, pad 1)                           [B,C,2,2]
  x1  = conv1x1(x0, w2, stride 2, pad 1)                 [B,C,2,2]
  x1  = (x1 - x1.min()) / (x1.max() - x1.min()) * 2
  x4  = (x - x.min()) / (x.max() - x.min())
  x44 = per-quadrant exposure adjust of x4 with gammas from x1
  y   = x + (x4 * (x.max()-x.min()) + x.min())

Key algebraic reductions baked in here:
  * The stride-2/pad-1 1x1 conv samples the zero padding at 3 of its 4
    output positions, so x1[:,:,0,0] = x1[:,:,0,1] = x1[:,:,1,0] = 0 and
    only x1[:,:,1,1] = w2 @ (conv3x3 output at (1,1)) carries data.
  * The conv3x3 output at (1,1) only reads taps (kh,kw) in {0,1}^2, i.e.
    v[b,d] = sum_{o} w2[d,o] * sum_{c,i,j} pool[b,c,i,j] * w1[o,c,i,j].
  * The min-max rescale of x1 is invariant to positive scaling, so the
    /16384 pooling normalization is dropped (v is 16384x the true value).

Per core (2 batches): partition p = b*64 + c; two streaming passes over x.
Pass 1: per-(b,c) quadrant sums (ScalarE activation-accumulate) and global
min/max via ONE fused custom DVE op per tile (AGC_MINMAX below).  Tiny
convs as 128x128 block-diagonal matmuls on the TensorEngine.  One 4-float
AllReduce(max) carries {-xmin, xmax, -vmin, vmax} across the 8 cores;
while it rendezvouses, prefetch DMAs pull the first pass-2 tiles.
Pass 2: normalize (VectorE 2x tensor_scalar), ln/exp exposure adjust on
ScalarE with both branches blended via per-partition scale/bias (the pow
branch is killed with bias=-1e30 when gamma<1, the log branch via a zero
coefficient otherwise; ln+exp share one activation-table set), and
y = x*sy + by reconstruction.  Wall time is DMA-bound in both passes
(~134 MB/core over ~2.9 TB/s chip HBM); measured ~450-460 us on silicon.
"""

import numpy as np

import concourse.bacc as bacc
import concourse.mybir as mybir
from concourse import dve_ops, masks, tile
from concourse.bass_utils import run_bass_kernel_spmd
from concourse.dve_spec import (AluOp, C0, Spec, Src0, Src1, eq, lower, maxx,
                                scan, select)
from concourse.dve_spec import _has_src1 as has_src1
from concourse.dve_uop import DveOpSpec

F32 = mybir.dt.float32
ALU = mybir.AluOpType
AF = mybir.ActivationFunctionType
AX = mybir.AxisListType

N_CORES = 8
INV_LN2 = float(1.0 / np.log(2.0))
NEG_BIG = -1.0e30
PRE_K_IN = 6  # pass-2 input tiles (16 rows) prefetched before the collective
KEEP_J = 0  # trailing pass-1 x tiles kept resident for pass-2 reuse

def _ref_agc_minmax(in0, in1, c0, c1, c2):
    # body[k] = in0[k] == in0[k] ? in0[k] : running_min(in1)[k] (NaN pad
    # marks the slot that emits the completed min); accum = max(body)
    x0 = in0.astype(np.float32).reshape(in0.shape[0], -1)
    x1 = in1.astype(np.float32).reshape(in0.shape[0], -1)
    smin = np.fmin.accumulate(np.where(np.isnan(x1), c0, x1), axis=-1)
    body = np.where(~np.isnan(x0), x0, smin)
    acc = np.fmax.reduce(body, axis=-1).reshape(-1, 1)
    return body, acc


# One 1x DVE pass yielding BOTH extrema of a [P, N] tile: stream the tile
# plus one trailing NaN pad element. body passes the raw element through
# (feeding the max accumulator) except at the pad slot, which emits the
# completed running min (NaN is identity for the DVE's minNum/maxNum ALU).
AGC_MINMAX = dve_ops.DveOp(
    "AGC_MINMAX",
    Spec(
        body=select(eq(Src0, Src0), Src0, scan(AluOp.MIN, Src1, init=C0)),
        accum=maxx,
        reference=_ref_agc_minmax,
    ),
    subdim=False,
    uops_sha={},
)


def _register_agc_minmax():
    if AGC_MINMAX.name in dve_ops._SUB_OPCODE_FOR_NAME:
        return
    dve_ops.OPS.append(AGC_MINMAX)
    dve_ops._SUB_OPCODE_FOR_NAME[AGC_MINMAX.name] = (
        dve_ops._CUSTOM_DVE_ROW_BASE + len(dve_ops.OPS) - 1
    )
    assert max(dve_ops._SUB_OPCODE_FOR_NAME.values()) < 0x20
    dve_ops.CUSTOM_DVE_SPECS[AGC_MINMAX.name] = AGC_MINMAX.spec
    # self-pin the uop shas (compile() raises on unpinned/drifted shas)
    for ver in ("v3", "v4"):
        spec = DveOpSpec(
            name=AGC_MINMAX.name,
            opcode=dve_ops.get_dve_sub_opcode(AGC_MINMAX.name),
            uops=lower(AGC_MINMAX.spec, ver=ver),
            rd1_en=has_src1(AGC_MINMAX.spec),
        )
        AGC_MINMAX.uops_sha[ver] = spec.sha(ver)


_ACT_SET = "natural_log_exp_and_others"  # holds ln+exp+copy: one table load


def _patch_act_tables():
    # The greedy table-set chooser pairs Ln with "natural_log" and Exp with
    # "exp_and_others", reloading tables (~1.3us) around every activation.
    # Every function this kernel uses lives in _ACT_SET, so blank out the
    # other sets (indices must be preserved — they are act_func_set_ids).
    if getattr(bacc, "_agc_act_patch", False):
        return
    orig = bacc.get_activation_tables

    def patched(arch):
        tabs = orig(arch)
        if not any(n == _ACT_SET for n in tabs):
            return tabs
        return {n: (fns if n == _ACT_SET else set()) for n, fns in tabs.items()}

    bacc.get_activation_tables = patched
    bacc._agc_act_patch = True


def build_kernel(B_sh=2, C=64, H=256, W=256, r1=16, r2=8, n_cores=N_CORES,
                 finalize=True):
    P = B_sh * C
    assert P == 128
    hw = W // 2
    hh = H // 2
    T1 = H // r1
    T2 = H // r2
    assert hh % r1 == 0 and hh % r2 == 0

    _register_agc_minmax()
    nc = bacc.Bacc(None, target_bir_lowering=False, debug=False)
    x_ext = nc.declare_dram_parameter("x", [B_sh, C, H, W], F32, isOutput=False)
    w1_ext = nc.declare_dram_parameter("w1", [C, C, 3, 3], F32, isOutput=False)
    w2_ext = nc.declare_dram_parameter("w2", [C, C, 1, 1], F32, isOutput=False)
    y_ext = nc.declare_dram_parameter("y", [B_sh, C, H, W], F32, isOutput=True)
    o_ext = nc.declare_dram_parameter("x44", [B_sh, C, H, W], F32, isOutput=True)

    xv = x_ext.ap().rearrange("b c h w -> (b c) h w")
    yv = y_ext.ap().rearrange("b c h w -> (b c) h w")
    ov = o_ext.ap().rearrange("b c h w -> (b c) h w")
    groups = [list(range(n_cores))]

    with tile.TileContext(nc) as tc:
        with (
            tc.tile_pool(name="const", bufs=1) as constp,
            tc.tile_pool(name="stats", bufs=1) as statp,
            tc.tile_pool(name="psum", bufs=1, space="PSUM") as psum,
            tc.tile_pool(name="dram", bufs=1, space="DRAM") as dram,
        ):
            # warm up the collective pipeline during pass 1: if the ~30us
            # cost of the first AllReduce is ring spin-up, pay it here where
            # the DMA stream hides it instead of in the mid-kernel window
            warm_in = dram.tile([1, 1], F32)
            warm_out = dram.tile([1, 1], F32)
            nc.gpsimd.dma_start(out=warm_in[:], in_=x_ext[0:1, 0, 0, 0:1])
            nc.gpsimd.collective_compute(
                "AllReduce", ALU.max, replica_groups=groups,
                ins=[warm_in[:].opt()], outs=[warm_out[:].opt()],
            )

            ident = constp.tile([P, P], F32)
            masks.make_identity(nc, ident[:])
            ones1 = constp.tile([1, P], F32)
            nc.gpsimd.memset(ones1[:], 1.0)

            w1sb = constp.tile([C, C * 9], F32)
            nc.sync.dma_start(
                out=w1sb[:], in_=w1_ext.ap().rearrange("o c kh kw -> o (c kh kw)")
            )
            w2sb = constp.tile([C, C], F32)
            nc.sync.dma_start(
                out=w2sb[:], in_=w2_ext.ap().rearrange("d o kh kw -> d (o kh kw)")
            )

            # Block-diagonal stationary weights: lhsT[(b',c), (b,o)] =
            # delta(b,b') * w1[o,c,tap] so K can stay on the (b,c) partitions.
            w1v = w1sb[:].rearrange("o (c k) -> o c k", k=9)
            w1blks = []
            for i, j in [(0, 0), (0, 1), (1, 0), (1, 1)]:
                tap = i * 3 + j
                trp = psum.tile([C, C], F32)
                nc.tensor.transpose(trp[:], w1v[:, :, tap], ident[0:C, 0:C])
                blk = constp.tile([P, P], F32)
                nc.vector.memset(blk[:], 0.0)
                nc.scalar.copy(out=blk[0:C, 0:C], in_=trp[:])
                nc.scalar.copy(out=blk[C:P, C:P], in_=trp[:])
                w1blks.append(blk)
            tr2 = psum.tile([C, C], F32)
            nc.tensor.transpose(tr2[:], w2sb[:], ident[0:C, 0:C])
            w2blk = constp.tile([P, P], F32)
            nc.vector.memset(w2blk[:], 0.0)
            nc.scalar.copy(out=w2blk[0:C, 0:C], in_=tr2[:])
            nc.scalar.copy(out=w2blk[C:P, C:P], in_=tr2[:])

            # ---------------- pass 1: stream x, gather stats ----------------
            # DVE: ONE fused pass per tile (AGC_MINMAX custom op, in place
            # over the tile + NaN pad column) -> max via accum_out, min in
            # the pad column.  ScalarE: left/right row sums via activation
            # accumulate (reads happen before the in-place DVE write).
            minp = statp.tile([P, T1], F32)
            maxp = statp.tile([P, T1], F32)
            sl = statp.tile([P, T1], F32)
            sr = statp.tile([P, T1], F32)
            N1 = r1 * W

            from contextlib import ExitStack

            keep_j = min(KEEP_J, T1)
            p2x_cm = tc.tile_pool(name="p2x", bufs=PRE_K_IN)
            p2x = p2x_cm.__enter__()
            es1 = ExitStack()
            p1x = es1.enter_context(tc.tile_pool(name="p1x", bufs=keep_j + 4))
            p1ascr = es1.enter_context(tc.tile_pool(name="p1ascr", bufs=1))
            keep = {}
            for t in range(T1):
                r0 = t * r1
                xt = p1x.tile([P, N1 + 1], F32)
                if t >= T1 - keep_j:
                    keep[t] = xt
                nc.sync.dma_start(out=xt[:, 0:N1], in_=xv[:, r0 : r0 + r1, :])
                nc.gpsimd.memset(xt[:, N1 : N1 + 1], float("nan"))
                xt3 = xt[:, 0:N1].rearrange("p (r w) -> p r w", w=W)
                a1 = p1ascr.tile([P, r1, W], F32)
                nc.scalar.activation(
                    out=a1[:, :, 0:hw], in_=xt3[:, :, 0:hw], func=AF.Copy,
                    accum_out=sl[:, t : t + 1],
                )
                nc.scalar.activation(
                    out=a1[:, :, hw:W], in_=xt3[:, :, hw:W], func=AF.Copy,
                    accum_out=sr[:, t : t + 1],
                )
                nc.vector._custom_dve(
                    AGC_MINMAX, out=xt[:], in0=xt[:], in1=xt[:],
                    s0=3.4e38, accum_out=maxp[:, t : t + 1],
                )
                nc.scalar.copy(out=minp[:, t : t + 1], in_=xt[:, N1 : N1 + 1])

            # Pass-2 iteration order interleaves top/bottom halves so the
            # heavier ScalarE work of bottom tiles (split exp + split blend)
            # spreads evenly instead of piling up in an ACT-bound tail.
            keep_row0 = (T1 - keep_j) * r1
            r2in = 2 * r2
            T2IN = H // r2in
            half = T2 // 2
            t_order = []
            for i in range(half):
                t_order += [i, half + i]
            ti_order = []
            for t in t_order:
                ti = t // 2
                if ti not in ti_order:
                    ti_order.append(ti)

            # prefetch the first pass-2 INPUT tiles (16 rows each; compute
            # consumes them in 8-row slices) into the collective window
            xts = {}
            for ti in ti_order[: min(PRE_K_IN, T2IN)]:
                if ti * r2in >= keep_row0:
                    continue
                xt = p2x.tile([P, r2in, W], F32, name="p2xt", tag="p2xt")
                nc.sync.dma_start(out=xt[:], in_=xv[:, ti * r2in : (ti + 1) * r2in, :])
                xts[ti] = xt
            es1.close()

            # ------------- finals + tiny convs + all-reduce ------------------
            ht = T1 // 2
            S = statp.tile([P, 4], F32)
            nc.vector.tensor_reduce(out=S[:, 0:1], in_=sl[:, 0:ht], axis=AX.X, op=ALU.add)
            nc.vector.tensor_reduce(out=S[:, 1:2], in_=sr[:, 0:ht], axis=AX.X, op=ALU.add)
            nc.vector.tensor_reduce(out=S[:, 2:3], in_=sl[:, ht:T1], axis=AX.X, op=ALU.add)
            nc.vector.tensor_reduce(out=S[:, 3:4], in_=sr[:, ht:T1], axis=AX.X, op=ALU.add)
            xminv = statp.tile([P, 1], F32)
            xmaxv = statp.tile([P, 1], F32)
            nc.vector.tensor_reduce(out=xminv[:], in_=minp[:], axis=AX.X, op=ALU.min)
            nc.vector.tensor_reduce(out=xmaxv[:], in_=maxp[:], axis=AX.X, op=ALU.max)

            qp = psum.tile([P, 1], F32)
            for k in range(4):
                nc.tensor.matmul(
                    qp[:], lhsT=w1blks[k][:], rhs=S[:, k : k + 1],
                    start=(k == 0), stop=(k == 3),
                )
            qsb = statp.tile([P, 1], F32)
            nc.scalar.copy(out=qsb[:], in_=qp[:])
            vp = psum.tile([P, 1], F32)
            nc.tensor.matmul(vp[:], lhsT=w2blk[:], rhs=qsb[:], start=True, stop=True)
            vsb = statp.tile([P, 1], F32)
            nc.scalar.copy(out=vsb[:], in_=vp[:])

            # single 4-float AllReduce(max): [-xmin, xmax, -vmin, vmax]
            pk = statp.tile([P, 4], F32)
            nc.vector.tensor_scalar(out=pk[:, 0:1], in0=xminv[:], scalar1=-1.0,
                                    scalar2=None, op0=ALU.mult)
            nc.vector.tensor_copy(out=pk[:, 1:2], in_=xmaxv[:])
            nc.vector.tensor_scalar(out=pk[:, 2:3], in0=vsb[:], scalar1=-1.0,
                                    scalar2=None, op0=ALU.mult)
            nc.vector.tensor_copy(out=pk[:, 3:4], in_=vsb[:])
            pkt = psum.tile([4, P], F32)
            nc.tensor.transpose(pkt[:], pk[:], ident[:])
            red4 = statp.tile([4, 1], F32)
            nc.vector.tensor_reduce(out=red4[:], in_=pkt[:], axis=AX.X, op=ALU.max)
            cc_in = dram.tile([4, 1], F32)
            cc_out = dram.tile([4, 1], F32)
            nc.gpsimd.dma_start(out=cc_in[:], in_=red4[:])
            nc.gpsimd.collective_compute(
                "AllReduce", ALU.max, replica_groups=groups,
                ins=[cc_in[:].opt()], outs=[cc_out[:].opt()],
            )
            gsb = statp.tile([1, 4], F32)
            nc.gpsimd.dma_start(out=gsb[:], in_=cc_out[:])
            gps = psum.tile([P, 4], F32)
            nc.tensor.matmul(gps[:], lhsT=ones1[:], rhs=gsb[:], start=True, stop=True)
            GX = statp.tile([P, 4], F32)  # cols: -x2, x3, -vmin_g, vmax_g
            nc.scalar.copy(out=GX[:], in_=gps[:])
            GV = GX[:, 2:4]

            def pvec(tag):
                return statp.tile([P, 1], F32, name=tag, tag=tag)

            c_x2 = pvec("c_x2")
            nc.vector.tensor_scalar(out=c_x2[:], in0=GX[:, 0:1], scalar1=-1.0,
                                    scalar2=None, op0=ALU.mult)
            c_r = pvec("c_r")
            nc.vector.tensor_tensor(out=c_r[:], in0=GX[:, 1:2], in1=GX[:, 0:1], op=ALU.add)
            c_invr = pvec("c_invr")
            nc.vector.reciprocal(out=c_invr[:], in_=c_r[:])
            c_negm0 = pvec("c_negm0")  # -m0 = max(0, -vmin_g)
            nc.vector.tensor_scalar(out=c_negm0[:], in0=GV[:, 0:1], scalar1=0.0,
                                    scalar2=None, op0=ALU.max)
            c_M0 = pvec("c_M0")
            nc.vector.tensor_scalar(out=c_M0[:], in0=GV[:, 1:2], scalar1=0.0,
                                    scalar2=None, op0=ALU.max)
            c_rng = pvec("c_rng")
            nc.vector.tensor_tensor(out=c_rng[:], in0=c_M0[:], in1=c_negm0[:], op=ALU.add)
            c_invg = pvec("c_invg")
            nc.vector.reciprocal(out=c_invg[:], in_=c_rng[:])
            c_tw = pvec("c_tw")
            nc.vector.tensor_scalar(out=c_tw[:], in0=c_invg[:], scalar1=2.0,
                                    scalar2=None, op0=ALU.mult)
            c_gabr = pvec("c_gabr")  # (v - m0) * 2/(M0-m0)
            nc.vector.tensor_scalar(out=c_gabr[:], in0=vsb[:], scalar1=c_negm0[:],
                                    scalar2=c_tw[:], op0=ALU.add, op1=ALU.mult)
            c_ga0 = pvec("c_ga0")  # (0 - m0) * 2/(M0-m0)
            nc.vector.tensor_tensor(out=c_ga0[:], in0=c_negm0[:], in1=c_tw[:], op=ALU.mult)
            c_mbr = pvec("c_mbr")
            nc.vector.tensor_scalar(out=c_mbr[:], in0=c_gabr[:], scalar1=1.0,
                                    scalar2=None, op0=ALU.is_lt)
            c_m0m = pvec("c_m0m")
            nc.vector.tensor_scalar(out=c_m0m[:], in0=c_ga0[:], scalar1=1.0,
                                    scalar2=None, op0=ALU.is_lt)
            c_lcbr = pvec("c_lcbr")  # mask * gamma / ln2
            nc.vector.scalar_tensor_tensor(out=c_lcbr[:], in0=c_gabr[:], scalar=INV_LN2,
                                           in1=c_mbr[:], op0=ALU.mult, op1=ALU.mult)
            c_lc0 = pvec("c_lc0")
            nc.vector.scalar_tensor_tensor(out=c_lc0[:], in0=c_ga0[:], scalar=INV_LN2,
                                           in1=c_m0m[:], op0=ALU.mult, op1=ALU.mult)
            c_pbbr = pvec("c_pbbr")  # -1e30 where log branch, else 0
            nc.vector.tensor_scalar(out=c_pbbr[:], in0=c_mbr[:], scalar1=NEG_BIG,
                                    scalar2=None, op0=ALU.mult)
            c_pb0 = pvec("c_pb0")
            nc.vector.tensor_scalar(out=c_pb0[:], in0=c_m0m[:], scalar1=NEG_BIG,
                                    scalar2=None, op0=ALU.mult)
            # y = x + ((x-x2)*inv_r)*r + x2 = x*sy + by with kap = r*inv_r,
            # sy = 1+kap, by = x2*(1-kap)  (kap is 1 +- 1ulp)
            c_kap = pvec("c_kap")
            nc.vector.tensor_tensor(out=c_kap[:], in0=c_r[:], in1=c_invr[:], op=ALU.mult)
            c_sy = pvec("c_sy")
            nc.vector.tensor_scalar(out=c_sy[:], in0=c_kap[:], scalar1=1.0,
                                    scalar2=None, op0=ALU.add)
            c_om = pvec("c_om")
            nc.vector.tensor_scalar(out=c_om[:], in0=c_kap[:], scalar1=-1.0,
                                    scalar2=1.0, op0=ALU.mult, op1=ALU.add)
            c_by = pvec("c_by")
            nc.vector.tensor_tensor(out=c_by[:], in0=c_x2[:], in1=c_om[:], op=ALU.mult)

            # ---------------- pass 2: stream x, emit y and x44 ----------------
            es2 = ExitStack()
            p2t = es2.enter_context(tc.tile_pool(name="p2t", bufs=3))
            p2a = es2.enter_context(tc.tile_pool(name="p2a", bufs=2))
            p2u = es2.enter_context(tc.tile_pool(name="p2u", bufs=2))
            p2g = es2.enter_context(tc.tile_pool(name="p2g", bufs=3))
            def issue_in(ti):
                # issue input-tile ti's DMA ahead of earlier tiles' output
                # DMAs so the in-order sync sequencer never parks an input
                # issue behind an output issue that waits on compute
                if ti >= T2IN or ti in xts or ti * r2in >= keep_row0:
                    return
                xt = p2x.tile([P, r2in, W], F32, name="p2xt", tag="p2xt")
                nc.sync.dma_start(out=xt[:], in_=xv[:, ti * r2in : ti * r2in + r2in, :])
                xts[ti] = xt

            FETCH_AHEAD = 5
            remaining = {}  # input tile -> uses left
            for t in t_order:
                remaining[t // 2] = remaining.get(t // 2, 0) + 1
            if True:
                for pos, t in enumerate(t_order):
                    r0 = t * r2
                    top = (r0 + r2) <= hh
                    ti = t // 2
                    issue_in(ti)
                    ahead = pos // 2 + FETCH_AHEAD
                    if ahead < len(ti_order):
                        issue_in(ti_order[ahead])
                    if r0 >= keep_row0:
                        kt = r0 // r1
                        kview = keep[kt][:, 0:N1].rearrange("p (r w) -> p r w", w=W)
                        xt = kview[:, r0 - kt * r1 : r0 - kt * r1 + r2, :]
                    else:
                        off = r0 - ti * r2in
                        remaining[ti] -= 1
                        xtile = xts[ti] if remaining[ti] else xts.pop(ti)
                        xt = xtile[:, off : off + r2, :]
                    tt = p2t.tile([P, r2, W], F32)  # t = (x - x2) / r
                    nc.vector.tensor_scalar(out=tt[:], in0=xt[:], scalar1=GX[:, 0:1],
                                            scalar2=c_invr[:], op0=ALU.add, op1=ALU.mult)
                    a_ = p2a.tile([P, r2, W], F32)  # ln(1 + t)
                    nc.scalar.activation(out=a_[:], in_=tt[:], func=AF.Ln, bias=1.0)
                    u_ = p2u.tile([P, r2, W], F32)  # ln(t)
                    nc.scalar.activation(out=u_[:], in_=tt[:], func=AF.Ln)
                    # y = x*sy + by into the t buffer (free once both LNs
                    # have read it) so y and its store don't wait on the
                    # ACT-dependent blend below
                    nc.vector.tensor_scalar(out=tt[:], in0=xt[:], scalar1=c_sy[:],
                                            scalar2=c_by[:], op0=ALU.mult, op1=ALU.add)
                    nc.sync.dma_start(out=yv[:, r0 : r0 + r2, :], in_=tt[:])
                    g_ = p2g.tile([P, r2, W], F32)  # exp(ga*ln t + pbias)
                    if top:
                        nc.scalar.activation(out=g_[:], in_=u_[:], func=AF.Exp,
                                             scale=c_ga0[:], bias=c_pb0[:])
                        nc.vector.scalar_tensor_tensor(
                            out=g_[:], in0=a_[:], scalar=c_lc0[:], in1=g_[:],
                            op0=ALU.mult, op1=ALU.add)
                    else:
                        nc.scalar.activation(out=g_[:, :, 0:hw], in_=u_[:, :, 0:hw],
                                             func=AF.Exp, scale=c_ga0[:], bias=c_pb0[:])
                        nc.scalar.activation(out=g_[:, :, hw:W], in_=u_[:, :, hw:W],
                                             func=AF.Exp, scale=c_gabr[:], bias=c_pbbr[:])
                        nc.vector.scalar_tensor_tensor(
                            out=g_[:, :, 0:hw], in0=a_[:, :, 0:hw], scalar=c_lc0[:],
                            in1=g_[:, :, 0:hw], op0=ALU.mult, op1=ALU.add)
                        nc.vector.scalar_tensor_tensor(
                            out=g_[:, :, hw:W], in0=a_[:, :, hw:W], scalar=c_lcbr[:],
                            in1=g_[:, :, hw:W], op0=ALU.mult, op1=ALU.add)
                    nc.sync.dma_start(out=ov[:, r0 : r0 + r2, :], in_=g_[:])
            es2.close()
            p2x_cm.__exit__(None, None, None)
    if finalize:
        _patch_act_tables()
        nc.finalize()
    return nc


_NC_CACHE = {}


def _get_nc(**kw):
    key = tuple(sorted(kw.items()))
    if key not in _NC_CACHE:
        _NC_CACHE[key] = build_kernel(**kw)
    return _NC_CACHE[key]


def kernel(x, w1, w2):
    x = np.ascontiguousarray(x, dtype=np.float32)
    w1 = np.ascontiguousarray(w1, dtype=np.float32)
    w2 = np.ascontiguousarray(w2, dtype=np.float32)
    B = x.shape[0]
    bs = B // N_CORES
    nc = _get_nc(B_sh=bs, C=x.shape[1], H=x.shape[2], W=x.shape[3])
    in_maps = [
        {"x": x[i * bs : (i + 1) * bs], "w1": w1, "w2": w2} for i in range(N_CORES)
    ]
    res = run_bass_kernel_spmd(nc, in_maps, core_ids=list(range(N_CORES)))
    y = np.concatenate([res.results[i]["y"] for i in range(N_CORES)], axis=0)
    x44 = np.concatenate([res.results[i]["x44"] for i in range(N_CORES)], axis=0)
    return y, x44


# revision 52
# speedup vs baseline: 1.0081x; 1.0081x over previous
"""Trainium2 Bass kernel for nn_AGCnet — 8-core batch-parallel.

Reference structure (B=16, C=64, H=W=256):
  x0  = AdaptiveAvgPool2d((2,2))(x)                      [B,C,2,2]
  x0  = conv3x3(x0, w1, pad 1)                           [B,C,2,2]
  x1  = conv1x1(x0, w2, stride 2, pad 1)                 [B,C,2,2]
  x1  = (x1 - x1.min()) / (x1.max() - x1.min()) * 2
  x4  = (x - x.min()) / (x.max() - x.min())
  x44 = per-quadrant exposure adjust of x4 with gammas from x1
  y   = x + (x4 * (x.max()-x.min()) + x.min())

Key algebraic reductions baked in here:
  * The stride-2/pad-1 1x1 conv samples the zero padding at 3 of its 4
    output positions, so x1[:,:,0,0] = x1[:,:,0,1] = x1[:,:,1,0] = 0 and
    only x1[:,:,1,1] = w2 @ (conv3x3 output at (1,1)) carries data.
  * The conv3x3 output at (1,1) only reads taps (kh,kw) in {0,1}^2, i.e.
    v[b,d] = sum_{o} w2[d,o] * sum_{c,i,j} pool[b,c,i,j] * w1[o,c,i,j].
  * The min-max rescale of x1 is invariant to positive scaling, so the
    /16384 pooling normalization is dropped (v is 16384x the true value).

Per core (2 batches): partition p = b*64 + c; two streaming passes over x.
Pass 1: per-(b,c) quadrant sums (ScalarE activation-accumulate) and global
min/max via ONE fused custom DVE op per tile (AGC_MINMAX below).  Tiny
convs as 128x128 block-diagonal matmuls on the TensorEngine.  One 4-float
AllReduce(max) carries {-xmin, xmax, -vmin, vmax} across the 8 cores;
while it rendezvouses, prefetch DMAs pull the first pass-2 tiles.
Pass 2: normalize (VectorE 2x tensor_scalar), ln/exp exposure adjust on
ScalarE with both branches blended via per-partition scale/bias (the pow
branch is killed with bias=-1e30 when gamma<1, the log branch via a zero
coefficient otherwise; ln+exp share one activation-table set), and
y = x*sy + by reconstruction.  Wall time is DMA-bound in both passes
(~134 MB/core over ~2.9 TB/s chip HBM); measured ~450-460 us on silicon.
"""

import numpy as np

import concourse.bacc as bacc
import concourse.mybir as mybir
from concourse import dve_ops, masks, tile
from concourse.bass_utils import run_bass_kernel_spmd
from concourse.dve_spec import (AluOp, C0, Spec, Src0, Src1, eq, lower, maxx,
                                scan, select)
from concourse.dve_spec import _has_src1 as has_src1
from concourse.dve_uop import DveOpSpec

F32 = mybir.dt.float32
ALU = mybir.AluOpType
AF = mybir.ActivationFunctionType
AX = mybir.AxisListType

N_CORES = 8
INV_LN2 = float(1.0 / np.log(2.0))
NEG_BIG = -1.0e30
PRE_K_IN = 5  # pass-2 input tiles (16 rows) prefetched before the collective
KEEP_J = 0  # trailing pass-1 x tiles kept resident for pass-2 reuse

def _ref_agc_minmax(in0, in1, c0, c1, c2):
    # body[k] = in0[k] == in0[k] ? in0[k] : running_min(in1)[k] (NaN pad
    # marks the slot that emits the completed min); accum = max(body)
    x0 = in0.astype(np.float32).reshape(in0.shape[0], -1)
    x1 = in1.astype(np.float32).reshape(in0.shape[0], -1)
    smin = np.fmin.accumulate(np.where(np.isnan(x1), c0, x1), axis=-1)
    body = np.where(~np.isnan(x0), x0, smin)
    acc = np.fmax.reduce(body, axis=-1).reshape(-1, 1)
    return body, acc


# One 1x DVE pass yielding BOTH extrema of a [P, N] tile: stream the tile
# plus one trailing NaN pad element. body passes the raw element through
# (feeding the max accumulator) except at the pad slot, which emits the
# completed running min (NaN is identity for the DVE's minNum/maxNum ALU).
AGC_MINMAX = dve_ops.DveOp(
    "AGC_MINMAX",
    Spec(
        body=select(eq(Src0, Src0), Src0, scan(AluOp.MIN, Src1, init=C0)),
        accum=maxx,
        reference=_ref_agc_minmax,
    ),
    subdim=False,
    uops_sha={},
)


def _register_agc_minmax():
    if AGC_MINMAX.name in dve_ops._SUB_OPCODE_FOR_NAME:
        return
    dve_ops.OPS.append(AGC_MINMAX)
    dve_ops._SUB_OPCODE_FOR_NAME[AGC_MINMAX.name] = (
        dve_ops._CUSTOM_DVE_ROW_BASE + len(dve_ops.OPS) - 1
    )
    assert max(dve_ops._SUB_OPCODE_FOR_NAME.values()) < 0x20
    dve_ops.CUSTOM_DVE_SPECS[AGC_MINMAX.name] = AGC_MINMAX.spec
    # self-pin the uop shas (compile() raises on unpinned/drifted shas)
    for ver in ("v3", "v4"):
        spec = DveOpSpec(
            name=AGC_MINMAX.name,
            opcode=dve_ops.get_dve_sub_opcode(AGC_MINMAX.name),
            uops=lower(AGC_MINMAX.spec, ver=ver),
            rd1_en=has_src1(AGC_MINMAX.spec),
        )
        AGC_MINMAX.uops_sha[ver] = spec.sha(ver)


_ACT_SET = "natural_log_exp_and_others"  # holds ln+exp+copy: one table load


def _patch_act_tables():
    # The greedy table-set chooser pairs Ln with "natural_log" and Exp with
    # "exp_and_others", reloading tables (~1.3us) around every activation.
    # Every function this kernel uses lives in _ACT_SET, so blank out the
    # other sets (indices must be preserved — they are act_func_set_ids).
    if getattr(bacc, "_agc_act_patch", False):
        return
    orig = bacc.get_activation_tables

    def patched(arch):
        tabs = orig(arch)
        if not any(n == _ACT_SET for n in tabs):
            return tabs
        return {n: (fns if n == _ACT_SET else set()) for n, fns in tabs.items()}

    bacc.get_activation_tables = patched
    bacc._agc_act_patch = True


def build_kernel(B_sh=2, C=64, H=256, W=256, r1=16, r2=8, n_cores=N_CORES,
                 finalize=True):
    P = B_sh * C
    assert P == 128
    hw = W // 2
    hh = H // 2
    T1 = H // r1
    T2 = H // r2
    assert hh % r1 == 0 and hh % r2 == 0

    _register_agc_minmax()
    nc = bacc.Bacc(None, target_bir_lowering=False, debug=False)
    x_ext = nc.declare_dram_parameter("x", [B_sh, C, H, W], F32, isOutput=False)
    w1_ext = nc.declare_dram_parameter("w1", [C, C, 3, 3], F32, isOutput=False)
    w2_ext = nc.declare_dram_parameter("w2", [C, C, 1, 1], F32, isOutput=False)
    y_ext = nc.declare_dram_parameter("y", [B_sh, C, H, W], F32, isOutput=True)
    o_ext = nc.declare_dram_parameter("x44", [B_sh, C, H, W], F32, isOutput=True)

    xv = x_ext.ap().rearrange("b c h w -> (b c) h w")
    yv = y_ext.ap().rearrange("b c h w -> (b c) h w")
    ov = o_ext.ap().rearrange("b c h w -> (b c) h w")
    groups = [list(range(n_cores))]

    with tile.TileContext(nc) as tc:
        with (
            tc.tile_pool(name="const", bufs=1) as constp,
            tc.tile_pool(name="stats", bufs=1) as statp,
            tc.tile_pool(name="psum", bufs=1, space="PSUM") as psum,
            tc.tile_pool(name="dram", bufs=1, space="DRAM") as dram,
        ):
            # warm up the collective pipeline during pass 1: if the ~30us
            # cost of the first AllReduce is ring spin-up, pay it here where
            # the DMA stream hides it instead of in the mid-kernel window
            warm_in = dram.tile([1, 1], F32)
            warm_out = dram.tile([1, 1], F32)
            nc.gpsimd.dma_start(out=warm_in[:], in_=x_ext[0:1, 0, 0, 0:1])
            nc.gpsimd.collective_compute(
                "AllReduce", ALU.max, replica_groups=groups,
                ins=[warm_in[:].opt()], outs=[warm_out[:].opt()],
            )

            ident = constp.tile([P, P], F32)
            masks.make_identity(nc, ident[:])
            ones1 = constp.tile([1, P], F32)
            nc.gpsimd.memset(ones1[:], 1.0)

            w1sb = constp.tile([C, C * 9], F32)
            nc.sync.dma_start(
                out=w1sb[:], in_=w1_ext.ap().rearrange("o c kh kw -> o (c kh kw)")
            )
            w2sb = constp.tile([C, C], F32)
            nc.sync.dma_start(
                out=w2sb[:], in_=w2_ext.ap().rearrange("d o kh kw -> d (o kh kw)")
            )

            # Block-diagonal stationary weights: lhsT[(b',c), (b,o)] =
            # delta(b,b') * w1[o,c,tap] so K can stay on the (b,c) partitions.
            w1v = w1sb[:].rearrange("o (c k) -> o c k", k=9)
            w1blks = []
            for i, j in [(0, 0), (0, 1), (1, 0), (1, 1)]:
                tap = i * 3 + j
                trp = psum.tile([C, C], F32)
                nc.tensor.transpose(trp[:], w1v[:, :, tap], ident[0:C, 0:C])
                blk = constp.tile([P, P], F32)
                nc.vector.memset(blk[:], 0.0)
                nc.scalar.copy(out=blk[0:C, 0:C], in_=trp[:])
                nc.scalar.copy(out=blk[C:P, C:P], in_=trp[:])
                w1blks.append(blk)
            tr2 = psum.tile([C, C], F32)
            nc.tensor.transpose(tr2[:], w2sb[:], ident[0:C, 0:C])
            w2blk = constp.tile([P, P], F32)
            nc.vector.memset(w2blk[:], 0.0)
            nc.scalar.copy(out=w2blk[0:C, 0:C], in_=tr2[:])
            nc.scalar.copy(out=w2blk[C:P, C:P], in_=tr2[:])

            # ---------------- pass 1: stream x, gather stats ----------------
            # DVE: ONE fused pass per tile (AGC_MINMAX custom op, in place
            # over the tile + NaN pad column) -> max via accum_out, min in
            # the pad column.  ScalarE: left/right row sums via activation
            # accumulate (reads happen before the in-place DVE write).
            minp = statp.tile([P, T1], F32)
            maxp = statp.tile([P, T1], F32)
            sl = statp.tile([P, T1], F32)
            sr = statp.tile([P, T1], F32)
            N1 = r1 * W

            from contextlib import ExitStack

            keep_j = min(KEEP_J, T1)
            p2x_cm = tc.tile_pool(name="p2x", bufs=PRE_K_IN + 1)
            p2x = p2x_cm.__enter__()
            es1 = ExitStack()
            p1x = es1.enter_context(tc.tile_pool(name="p1x", bufs=keep_j + 4))
            p1ascr = es1.enter_context(tc.tile_pool(name="p1ascr", bufs=1))
            keep = {}
            for t in range(T1):
                r0 = t * r1
                xt = p1x.tile([P, N1 + 1], F32)
                if t >= T1 - keep_j:
                    keep[t] = xt
                nc.sync.dma_start(out=xt[:, 0:N1], in_=xv[:, r0 : r0 + r1, :])
                nc.gpsimd.memset(xt[:, N1 : N1 + 1], float("nan"))
                xt3 = xt[:, 0:N1].rearrange("p (r w) -> p r w", w=W)
                a1 = p1ascr.tile([P, r1, W], F32)
                nc.scalar.activation(
                    out=a1[:, :, 0:hw], in_=xt3[:, :, 0:hw], func=AF.Copy,
                    accum_out=sl[:, t : t + 1],
                )
                nc.scalar.activation(
                    out=a1[:, :, hw:W], in_=xt3[:, :, hw:W], func=AF.Copy,
                    accum_out=sr[:, t : t + 1],
                )
                nc.vector._custom_dve(
                    AGC_MINMAX, out=xt[:], in0=xt[:], in1=xt[:],
                    s0=3.4e38, accum_out=maxp[:, t : t + 1],
                )
                nc.scalar.copy(out=minp[:, t : t + 1], in_=xt[:, N1 : N1 + 1])

            # Pass-2 iteration order interleaves top/bottom halves so the
            # heavier ScalarE work of bottom tiles (split exp + split blend)
            # spreads evenly instead of piling up in an ACT-bound tail.
            keep_row0 = (T1 - keep_j) * r1
            r2in = 2 * r2
            T2IN = H // r2in
            half = T2 // 2
            t_order = []
            for i in range(half):
                t_order += [i, half + i]
            ti_order = []
            for t in t_order:
                ti = t // 2
                if ti not in ti_order:
                    ti_order.append(ti)

            # prefetch the first pass-2 INPUT tiles (16 rows each; compute
            # consumes them in 8-row slices) into the collective window
            xts = {}
            for ti in ti_order[: min(PRE_K_IN, T2IN)]:
                if ti * r2in >= keep_row0:
                    continue
                xt = p2x.tile([P, r2in, W], F32, name="p2xt", tag="p2xt")
                nc.sync.dma_start(out=xt[:], in_=xv[:, ti * r2in : (ti + 1) * r2in, :])
                xts[ti] = xt
            es1.close()

            # ------------- finals + tiny convs + all-reduce ------------------
            ht = T1 // 2
            S = statp.tile([P, 4], F32)
            nc.vector.tensor_reduce(out=S[:, 0:1], in_=sl[:, 0:ht], axis=AX.X, op=ALU.add)
            nc.vector.tensor_reduce(out=S[:, 1:2], in_=sr[:, 0:ht], axis=AX.X, op=ALU.add)
            nc.vector.tensor_reduce(out=S[:, 2:3], in_=sl[:, ht:T1], axis=AX.X, op=ALU.add)
            nc.vector.tensor_reduce(out=S[:, 3:4], in_=sr[:, ht:T1], axis=AX.X, op=ALU.add)
            xminv = statp.tile([P, 1], F32)
            xmaxv = statp.tile([P, 1], F32)
            nc.vector.tensor_reduce(out=xminv[:], in_=minp[:], axis=AX.X, op=ALU.min)
            nc.vector.tensor_reduce(out=xmaxv[:], in_=maxp[:], axis=AX.X, op=ALU.max)

            qp = psum.tile([P, 1], F32)
            for k in range(4):
                nc.tensor.matmul(
                    qp[:], lhsT=w1blks[k][:], rhs=S[:, k : k + 1],
                    start=(k == 0), stop=(k == 3),
                )
            qsb = statp.tile([P, 1], F32)
            nc.scalar.copy(out=qsb[:], in_=qp[:])
            vp = psum.tile([P, 1], F32)
            nc.tensor.matmul(vp[:], lhsT=w2blk[:], rhs=qsb[:], start=True, stop=True)
            vsb = statp.tile([P, 1], F32)
            nc.scalar.copy(out=vsb[:], in_=vp[:])

            # single 4-float AllReduce(max): [-xmin, xmax, -vmin, vmax]
            pk = statp.tile([P, 4], F32)
            nc.vector.tensor_scalar(out=pk[:, 0:1], in0=xminv[:], scalar1=-1.0,
                                    scalar2=None, op0=ALU.mult)
            nc.vector.tensor_copy(out=pk[:, 1:2], in_=xmaxv[:])
            nc.vector.tensor_scalar(out=pk[:, 2:3], in0=vsb[:], scalar1=-1.0,
                                    scalar2=None, op0=ALU.mult)
            nc.vector.tensor_copy(out=pk[:, 3:4], in_=vsb[:])
            pkt = psum.tile([4, P], F32)
            nc.tensor.transpose(pkt[:], pk[:], ident[:])
            red4 = statp.tile([4, 1], F32)
            nc.vector.tensor_reduce(out=red4[:], in_=pkt[:], axis=AX.X, op=ALU.max)
            cc_in = dram.tile([4, 1], F32)
            cc_out = dram.tile([4, 1], F32)
            nc.gpsimd.dma_start(out=cc_in[:], in_=red4[:])
            nc.gpsimd.collective_compute(
                "AllReduce", ALU.max, replica_groups=groups,
                ins=[cc_in[:].opt()], outs=[cc_out[:].opt()],
            )
            gsb = statp.tile([1, 4], F32)
            nc.gpsimd.dma_start(out=gsb[:], in_=cc_out[:])
            gps = psum.tile([P, 4], F32)
            nc.tensor.matmul(gps[:], lhsT=ones1[:], rhs=gsb[:], start=True, stop=True)
            GX = statp.tile([P, 4], F32)  # cols: -x2, x3, -vmin_g, vmax_g
            nc.scalar.copy(out=GX[:], in_=gps[:])
            GV = GX[:, 2:4]

            def pvec(tag):
                return statp.tile([P, 1], F32, name=tag, tag=tag)

            c_x2 = pvec("c_x2")
            nc.vector.tensor_scalar(out=c_x2[:], in0=GX[:, 0:1], scalar1=-1.0,
                                    scalar2=None, op0=ALU.mult)
            c_r = pvec("c_r")
            nc.vector.tensor_tensor(out=c_r[:], in0=GX[:, 1:2], in1=GX[:, 0:1], op=ALU.add)
            c_invr = pvec("c_invr")
            nc.vector.reciprocal(out=c_invr[:], in_=c_r[:])
            c_negm0 = pvec("c_negm0")  # -m0 = max(0, -vmin_g)
            nc.vector.tensor_scalar(out=c_negm0[:], in0=GV[:, 0:1], scalar1=0.0,
                                    scalar2=None, op0=ALU.max)
            c_M0 = pvec("c_M0")
            nc.vector.tensor_scalar(out=c_M0[:], in0=GV[:, 1:2], scalar1=0.0,
                                    scalar2=None, op0=ALU.max)
            c_rng = pvec("c_rng")
            nc.vector.tensor_tensor(out=c_rng[:], in0=c_M0[:], in1=c_negm0[:], op=ALU.add)
            c_invg = pvec("c_invg")
            nc.vector.reciprocal(out=c_invg[:], in_=c_rng[:])
            c_tw = pvec("c_tw")
            nc.vector.tensor_scalar(out=c_tw[:], in0=c_invg[:], scalar1=2.0,
                                    scalar2=None, op0=ALU.mult)
            c_gabr = pvec("c_gabr")  # (v - m0) * 2/(M0-m0)
            nc.vector.tensor_scalar(out=c_gabr[:], in0=vsb[:], scalar1=c_negm0[:],
                                    scalar2=c_tw[:], op0=ALU.add, op1=ALU.mult)
            c_ga0 = pvec("c_ga0")  # (0 - m0) * 2/(M0-m0)
            nc.vector.tensor_tensor(out=c_ga0[:], in0=c_negm0[:], in1=c_tw[:], op=ALU.mult)
            c_mbr = pvec("c_mbr")
            nc.vector.tensor_scalar(out=c_mbr[:], in0=c_gabr[:], scalar1=1.0,
                                    scalar2=None, op0=ALU.is_lt)
            c_m0m = pvec("c_m0m")
            nc.vector.tensor_scalar(out=c_m0m[:], in0=c_ga0[:], scalar1=1.0,
                                    scalar2=None, op0=ALU.is_lt)
            c_lcbr = pvec("c_lcbr")  # mask * gamma / ln2
            nc.vector.scalar_tensor_tensor(out=c_lcbr[:], in0=c_gabr[:], scalar=INV_LN2,
                                           in1=c_mbr[:], op0=ALU.mult, op1=ALU.mult)
            c_lc0 = pvec("c_lc0")
            nc.vector.scalar_tensor_tensor(out=c_lc0[:], in0=c_ga0[:], scalar=INV_LN2,
                                           in1=c_m0m[:], op0=ALU.mult, op1=ALU.mult)
            c_pbbr = pvec("c_pbbr")  # -1e30 where log branch, else 0
            nc.vector.tensor_scalar(out=c_pbbr[:], in0=c_mbr[:], scalar1=NEG_BIG,
                                    scalar2=None, op0=ALU.mult)
            c_pb0 = pvec("c_pb0")
            nc.vector.tensor_scalar(out=c_pb0[:], in0=c_m0m[:], scalar1=NEG_BIG,
                                    scalar2=None, op0=ALU.mult)
            # y = x + ((x-x2)*inv_r)*r + x2 = x*sy + by with kap = r*inv_r,
            # sy = 1+kap, by = x2*(1-kap)  (kap is 1 +- 1ulp)
            c_kap = pvec("c_kap")
            nc.vector.tensor_tensor(out=c_kap[:], in0=c_r[:], in1=c_invr[:], op=ALU.mult)
            c_sy = pvec("c_sy")
            nc.vector.tensor_scalar(out=c_sy[:], in0=c_kap[:], scalar1=1.0,
                                    scalar2=None, op0=ALU.add)
            c_om = pvec("c_om")
            nc.vector.tensor_scalar(out=c_om[:], in0=c_kap[:], scalar1=-1.0,
                                    scalar2=1.0, op0=ALU.mult, op1=ALU.add)
            c_by = pvec("c_by")
            nc.vector.tensor_tensor(out=c_by[:], in0=c_x2[:], in1=c_om[:], op=ALU.mult)

            # ---------------- pass 2: stream x, emit y and x44 ----------------
            es2 = ExitStack()
            p2t = es2.enter_context(tc.tile_pool(name="p2t", bufs=3))
            p2a = es2.enter_context(tc.tile_pool(name="p2a", bufs=2))
            p2u = es2.enter_context(tc.tile_pool(name="p2u", bufs=2))
            p2g = es2.enter_context(tc.tile_pool(name="p2g", bufs=3))
            def issue_in(ti):
                # issue input-tile ti's DMA ahead of earlier tiles' output
                # DMAs so the in-order sync sequencer never parks an input
                # issue behind an output issue that waits on compute
                if ti >= T2IN or ti in xts or ti * r2in >= keep_row0:
                    return
                xt = p2x.tile([P, r2in, W], F32, name="p2xt", tag="p2xt")
                nc.sync.dma_start(out=xt[:], in_=xv[:, ti * r2in : ti * r2in + r2in, :])
                xts[ti] = xt

            FETCH_AHEAD = 5
            remaining = {}  # input tile -> uses left
            for t in t_order:
                remaining[t // 2] = remaining.get(t // 2, 0) + 1
            if True:
                for pos, t in enumerate(t_order):
                    r0 = t * r2
                    top = (r0 + r2) <= hh
                    ti = t // 2
                    issue_in(ti)
                    ahead = pos // 2 + FETCH_AHEAD
                    if ahead < len(ti_order):
                        issue_in(ti_order[ahead])
                    if r0 >= keep_row0:
                        kt = r0 // r1
                        kview = keep[kt][:, 0:N1].rearrange("p (r w) -> p r w", w=W)
                        xt = kview[:, r0 - kt * r1 : r0 - kt * r1 + r2, :]
                    else:
                        off = r0 - ti * r2in
                        remaining[ti] -= 1
                        xtile = xts[ti] if remaining[ti] else xts.pop(ti)
                        xt = xtile[:, off : off + r2, :]
                    tt = p2t.tile([P, r2, W], F32)  # t = (x - x2) / r
                    nc.vector.tensor_scalar(out=tt[:], in0=xt[:], scalar1=GX[:, 0:1],
                                            scalar2=c_invr[:], op0=ALU.add, op1=ALU.mult)
                    a_ = p2a.tile([P, r2, W], F32)  # ln(1 + t)
                    nc.scalar.activation(out=a_[:], in_=tt[:], func=AF.Ln, bias=1.0)
                    u_ = p2u.tile([P, r2, W], F32)  # ln(t)
                    nc.scalar.activation(out=u_[:], in_=tt[:], func=AF.Ln)
                    # y = x*sy + by into the t buffer (free once both LNs
                    # have read it) so y and its store don't wait on the
                    # ACT-dependent blend below
                    nc.vector.tensor_scalar(out=tt[:], in0=xt[:], scalar1=c_sy[:],
                                            scalar2=c_by[:], op0=ALU.mult, op1=ALU.add)
                    nc.sync.dma_start(out=yv[:, r0 : r0 + r2, :], in_=tt[:])
                    g_ = p2g.tile([P, r2, W], F32)  # exp(ga*ln t + pbias)
                    if top:
                        nc.scalar.activation(out=g_[:], in_=u_[:], func=AF.Exp,
                                             scale=c_ga0[:], bias=c_pb0[:])
                        nc.vector.scalar_tensor_tensor(
                            out=g_[:], in0=a_[:], scalar=c_lc0[:], in1=g_[:],
                            op0=ALU.mult, op1=ALU.add)
                    else:
                        nc.scalar.activation(out=g_[:, :, 0:hw], in_=u_[:, :, 0:hw],
                                             func=AF.Exp, scale=c_ga0[:], bias=c_pb0[:])
                        nc.scalar.activation(out=g_[:, :, hw:W], in_=u_[:, :, hw:W],
                                             func=AF.Exp, scale=c_gabr[:], bias=c_pbbr[:])
                        nc.vector.scalar_tensor_tensor(
                            out=g_[:, :, 0:hw], in0=a_[:, :, 0:hw], scalar=c_lc0[:],
                            in1=g_[:, :, 0:hw], op0=ALU.mult, op1=ALU.add)
                        nc.vector.scalar_tensor_tensor(
                            out=g_[:, :, hw:W], in0=a_[:, :, hw:W], scalar=c_lcbr[:],
                            in1=g_[:, :, hw:W], op0=ALU.mult, op1=ALU.add)
                    nc.sync.dma_start(out=ov[:, r0 : r0 + r2, :], in_=g_[:])
            es2.close()
            p2x_cm.__exit__(None, None, None)
    if finalize:
        _patch_act_tables()
        nc.finalize()
    return nc


_NC_CACHE = {}


def _get_nc(**kw):
    key = tuple(sorted(kw.items()))
    if key not in _NC_CACHE:
        _NC_CACHE[key] = build_kernel(**kw)
    return _NC_CACHE[key]


def kernel(x, w1, w2):
    x = np.ascontiguousarray(x, dtype=np.float32)
    w1 = np.ascontiguousarray(w1, dtype=np.float32)
    w2 = np.ascontiguousarray(w2, dtype=np.float32)
    B = x.shape[0]
    bs = B // N_CORES
    nc = _get_nc(B_sh=bs, C=x.shape[1], H=x.shape[2], W=x.shape[3])
    in_maps = [
        {"x": x[i * bs : (i + 1) * bs], "w1": w1, "w2": w2} for i in range(N_CORES)
    ]
    res = run_bass_kernel_spmd(nc, in_maps, core_ids=list(range(N_CORES)))
    y = np.concatenate([res.results[i]["y"] for i in range(N_CORES)], axis=0)
    x44 = np.concatenate([res.results[i]["x44"] for i in range(N_CORES)], axis=0)
    return y, x44
